# revision 5
# baseline (speedup 1.0000x reference)
"""Trainium2 Bass kernel for nn_ConditionalSplineSQ2D.

Math:
  out[b] = sum_{g,h,c} coeffs[g,h,c] * p[b,g,h,ii_c] * p[b,g,h,jj_c]
         = sum_{cells} p_cell^T S_cell p_cell            (S_cell symmetric 8x8)

Two per-group forms, assigned to balance all engines near the ~22us
DMA roofline for the 8MB fp16 per-core input:
  EM (eigen+mm2):   T = V^T p (PE), q = T*T (ACT square, fp16),
                    acc += lam^T q (PE mm2, PSUM-accumulated)
  AD (direct+add):  T = S^T p (PE), z = T .* p (DVE mult, fp16),
                    Zacc += z     (fp16 adds on DVE and GpSimd,
                                   two independent accumulator chains)

The AD class removes 28 groups' worth of mm2 streaming from the PE
(the baseline re-streams the whole Q tensor through the PE and is
PE-bound at ~26us); their accumulation lands on the DVE (2 elem/cyc
fp16 adds) and the otherwise-idle GpSimd.  Band evacuations and the
accumulator folds also run on GpSimd.

mm1 uses the 16x 32x32 PE sub-array tiling: each group's block-diag
stationary is 4 independent 32x32 tiles (4 cells each).  AD groups
must sit on the diagonal sub-arrays (c=r) so the PSUM partition layout
matches pt's layout for the elementwise multiply; EM groups rotate
to column band (r+sigma)%4 with sigma in 1..3.

pt DMA chunks alternate between the Sync and Scalar HWDGE rings so the
~650ns triggers issue in parallel, chunk boundaries align with wave
boundaries, and the first chunks are small so mm1 starts ~2.5us after
the window opens.  The last waves are AD so the kernel tail is just
the final adds + output DMA, not an ACT->mm2->evac chain.

Sharding: pure data parallel over batch (512 per core x 8 cores); the
4 mm2 accumulator rows plus the two 128-row fp16 Zacc tensors are
summed on host.
"""

import numpy as np

B, G, P = 4096, 31, 8
NCORES = 8
NB = B // NCORES  # 512 batches per core
CELLS = G * G  # 961
PARTS = 128
GROUP_CELLS = 16
NGROUPS = -(-CELLS // GROUP_CELLS)  # 61
CELLS_PAD = NGROUPS * GROUP_CELLS  # 976
WAVE = 2  # groups per wave (one PSUM tile per wave)
NWAVES = -(-NGROUPS // WAVE)  # 31 (last wave has 1 group)

# 14 AD waves (28 groups); the rest (33 groups) are EM.  Final double
# waves are AD so the kernel tail is short; wave 30 (single group) is EM.
_AD_WAVES = {2, 4, 6, 9, 11, 13, 15, 17, 19, 21, 23, 25, 28, 29}

PT_CHUNKS = [2, 2, 4, 6, 8, 8, 8, 8, 8, 7]  # pt DMA chunks (in groups)
assert sum(PT_CHUNKS) == NGROUPS
WL_CHUNKS = [5, 24, NGROUPS - 29]  # wl chunks (in groups); lam rides chunk 0
WL_LAM = NGROUPS  # lam columns at the front of the wl tensor

MM2_LAG = 3  # waves between a drain and its mm2


def _wave_class(w):
    return "AD" if w in _AD_WAVES else "EM"


def _group_form(g):
    return "A" if _wave_class(g // WAVE) == "AD" else "E"


def _sigma(g):
    """Column-band rotation for EM groups; 1..3 keeps them off the
    diagonal sub-arrays, which AD groups and mm2 bands already load."""
    return 1 + (g % 3)


def _col_band(g, r):
    """PE column band for tile r of group g (rotated for E, diagonal for A)."""
    return (r + _sigma(g)) % 4 if _group_form(g) == "E" else r


_EM_GROUPS = [g for g in range(NGROUPS) if _group_form(g) == "E"]
_N_AD_WAVES = len(_AD_WAVES)
_nc_cache = {}


def _build_nc():
    import concourse.mybir as mybir
    import concourse.tile as tile
    from concourse import bacc

    nc = bacc.Bacc()
    pt_d = nc.dram_tensor(
        "pt", [PARTS, NGROUPS * NB], mybir.dt.float16, kind="ExternalInput"
    )
    wl_d = nc.dram_tensor(
        "wl", [PARTS, WL_LAM + NGROUPS * 32], mybir.dt.float16, kind="ExternalInput"
    )
    out_d = nc.dram_tensor("out", [4, NB], mybir.dt.float32, kind="ExternalOutput")
    zacd_d = nc.dram_tensor("zacd", [PARTS, NB], mybir.dt.float16,
                            kind="ExternalOutput")
    zacg_d = nc.dram_tensor("zacg", [PARTS, NB], mybir.dt.float16,
                            kind="ExternalOutput")

    with tile.TileContext(nc) as tc:
        with (
            tc.tile_pool(name="const", bufs=1) as cpool,
            tc.tile_pool(name="wlp", bufs=len(WL_CHUNKS)) as wpool,
            tc.tile_pool(name="ptp", bufs=len(PT_CHUNKS)) as ppool,
            tc.tile_pool(name="qp", bufs=6) as qpool,
            tc.tile_pool(name="zp", bufs=3) as zpool,
            tc.tile_pool(name="psp", bufs=3, space="PSUM") as pspool,
            tc.tile_pool(name="accp", bufs=1, space="PSUM") as apool,
        ):
            acc = apool.tile([PARTS, NB], mybir.dt.float32)
            out_sb = cpool.tile([PARTS, NB], mybir.dt.float32)
            # two independent fp16 accumulator chains (ping-pong pairs)
            zaccd = [
                cpool.tile([PARTS, WAVE * NB], mybir.dt.float16, name=f"zaccd{i}",
                           tag=f"zaccd{i}")
                for i in range(2)
            ]
            zaccg = [
                cpool.tile([PARTS, WAVE * NB], mybir.dt.float16, name=f"zaccg{i}",
                           tag=f"zaccg{i}")
                for i in range(2)
            ]
            zoutd = cpool.tile([PARTS, NB], mybir.dt.float16)
            zoutg = cpool.tile([PARTS, NB], mybir.dt.float16)
            nc.gpsimd.memset(zaccd[0][:, :], 0.0)
            nc.gpsimd.memset(zaccg[0][:, :], 0.0)

            # ---- input DMAs ----
            # pt chunks alternate Sync/Scalar HWDGE rings so triggers
            # issue in parallel; the small wl chunk 0 (lam + 5 groups)
            # goes first on Scalar so wave 0 can start immediately.
            wtiles = []  # (tile, first_group, ngroups)
            wl_reqs = []
            g0 = 0
            for ci, ch in enumerate(WL_CHUNKS):
                ncols = (WL_LAM if ci == 0 else 0) + ch * 32
                wt = wpool.tile(
                    [PARTS, WL_LAM + max(WL_CHUNKS) * 32], mybir.dt.float16, tag="wl"
                )
                src0 = (WL_LAM if ci > 0 else 0) + g0 * 32
                wl_reqs.append((wt, ncols, src0))
                wtiles.append((wt, g0, ch))
                g0 += ch
            lam_sb = wtiles[0][0]  # cols [0, WL_LAM) of chunk 0
            wt0, nc0, s0 = wl_reqs[0]
            nc.scalar.dma_start(out=wt0[:, :nc0], in_=wl_d[:, s0:s0 + nc0])

            ptiles = []
            group_pt = {}  # group -> (tile_idx, offset_in_chunk)
            g0 = 0
            for ci, ch in enumerate(PT_CHUNKS):
                pt = ppool.tile([PARTS, max(PT_CHUNKS) * NB], mybir.dt.float16,
                                tag="pt")
                eng = nc.sync if ci % 2 == 0 else nc.scalar
                eng.dma_start(
                    out=pt[:, : ch * NB],
                    in_=pt_d[:, g0 * NB : (g0 + ch) * NB],
                )
                ptiles.append(pt)
                for k in range(ch):
                    group_pt[g0 + k] = (ci, k)
                g0 += ch
                if ci == 1:
                    wt1, nc1, s1 = wl_reqs[1]
                    nc.sync.dma_start(out=wt1[:, :nc1], in_=wl_d[:, s1:s1 + nc1])
                if ci == 3:
                    wt2, nc2, s2 = wl_reqs[2]
                    nc.sync.dma_start(out=wt2[:, :nc2], in_=wl_d[:, s2:s2 + nc2])

            def wt_slice(g, r):
                for wt, wg0, wch in wtiles:
                    if wg0 <= g < wg0 + wch:
                        c0 = (WL_LAM if wg0 == 0 else 0) + (g - wg0) * 32
                        return wt[32 * r : 32 * r + 32, c0 : c0 + 32]
                raise AssertionError(g)

            # ---- mm2 band bookkeeping (EM groups only) ----
            band_of = {g: i % 4 for i, g in enumerate(_EM_GROUPS)}
            first_in_band = {}
            last_in_band = {}
            for g in _EM_GROUPS:
                j = band_of[g]
                first_in_band.setdefault(j, g)
                last_in_band[j] = g

            q_slices = {}  # EM group -> (q_tile, slot)
            em_emitted = [0]  # index into _EM_GROUPS

            def evacuate_band(j):
                # acc row 32j -> out_sb row 32j (DMA remaps partitions).
                # GpSimd cannot read PSUM, so these stay on ACT/DVE.
                if j % 2:
                    nc.scalar.copy(
                        out_sb[32 * j : 32 * j + 1, :], acc[32 * j : 32 * j + 1, :]
                    )
                else:
                    nc.vector.tensor_copy(
                        out_sb[32 * j : 32 * j + 1, :], acc[32 * j : 32 * j + 1, :]
                    )

            def emit_mm2(limit_group):
                """Emit mm2 for EM groups < limit_group (in EM order)."""
                while em_emitted[0] < len(_EM_GROUPS):
                    g = _EM_GROUPS[em_emitted[0]]
                    if g >= limit_group:
                        return
                    qt, slot = q_slices.pop(g)
                    j = band_of[g]
                    nc.tensor.matmul(
                        acc[32 * j : 32 * j + 1, :],
                        lam_sb[:, g : g + 1],
                        qt[:, slot * NB : (slot + 1) * NB],
                        start=(g == first_in_band[j]),
                        stop=(g == last_in_band[j]),
                        tile_position=(0, 32 * j),
                    )
                    if g == last_in_band[j]:
                        evacuate_band(j)
                    em_emitted[0] += 1

            # ---- pipeline over waves ----
            ad_idx = [0]  # AD wave counter (chain assignment alternates)
            dctr = [0]  # adds emitted on the DVE chain
            gctr = [0]  # adds emitted on the GpSimd chain

            for w in range(NWAVES):
                wg0 = w * WAVE
                ng = min(WAVE, NGROUPS - wg0)
                cls = _wave_class(w)
                psT = pspool.tile([PARTS, WAVE * NB], mybir.dt.float32, tag="psT")
                for k in range(ng):
                    g = wg0 + k
                    ci, off = group_pt[g]
                    for r in range(4):
                        c = _col_band(g, r)
                        nc.tensor.matmul(
                            psT[32 * c : 32 * c + 32, k * NB : (k + 1) * NB],
                            wt_slice(g, r),
                            ptiles[ci][32 * r : 32 * r + 32,
                                       off * NB : (off + 1) * NB],
                            start=True,
                            stop=True,
                            tile_position=(32 * r, 32 * c),
                        )
                if cls == "EM":
                    # one fused ACT square per wave; mm2 follows (lagged)
                    q = qpool.tile([PARTS, WAVE * NB], mybir.dt.float16, tag="q")
                    nc.scalar.square(q[:, : ng * NB], psT[:, : ng * NB])
                    for k in range(ng):
                        q_slices[wg0 + k] = (q, k)
                else:
                    # direct drain on DVE (split at pt-chunk boundaries),
                    # then one fused fp16 add into the wave's chain
                    z = zpool.tile([PARTS, WAVE * NB], mybir.dt.float16, tag="z")
                    k = 0
                    while k < ng:
                        ci, off = group_pt[wg0 + k]
                        k1 = k + 1
                        while k1 < ng and group_pt[wg0 + k1] == (ci, off + k1 - k):
                            k1 += 1
                        nc.vector.tensor_mul(
                            z[:, k * NB : k1 * NB],
                            psT[:, k * NB : k1 * NB],
                            ptiles[ci][:, off * NB : (off + k1 - k) * NB],
                        )
                        k = k1
                    with nc.allow_low_precision(reason="fp16 Zacc, ~7 adds/chain"):
                        if ad_idx[0] % 2 == 0:
                            src = zaccd[dctr[0] % 2]
                            dst = zaccd[(dctr[0] + 1) % 2]
                            nc.vector.tensor_add(dst[:, :], src[:, :], z[:, :])
                            dctr[0] += 1
                        else:
                            src = zaccg[gctr[0] % 2]
                            dst = zaccg[(gctr[0] + 1) % 2]
                            nc.gpsimd.tensor_add(dst[:, :], src[:, :], z[:, :])
                            gctr[0] += 1
                    ad_idx[0] += 1
                # Lag mm2 behind the newest drain: matmuls start in strict
                # pc order, so an mm2 whose q drain is pending at the
                # PE-queue head stalls every later mm1 behind it.
                emit_mm2(max(0, wg0 - MM2_LAG * WAVE))
            emit_mm2(NGROUPS)

            # fold each [128, 2*NB] chain to [128, NB] and ship
            zd = zaccd[dctr[0] % 2]
            zg = zaccg[gctr[0] % 2]
            with nc.allow_low_precision(reason="fp16 Zacc fold"):
                nc.vector.tensor_add(zoutd[:, :], zd[:, :NB], zd[:, NB:])
                nc.gpsimd.tensor_add(zoutg[:, :], zg[:, :NB], zg[:, NB:])
            nc.sync.dma_start(out=zacd_d[:, :], in_=zoutd[:, :])
            nc.scalar.dma_start(out=zacg_d[:, :], in_=zoutg[:, :])
            # single strided DMA for the 4 mm2 partial-sum rows
            nc.sync.dma_start(out=out_d[:, :], in_=out_sb[0:97:32, :])
    if not nc.is_finalized():
        nc.finalize()
    return nc


def _get_nc():
    if "nc" not in _nc_cache:
        _nc_cache["nc"] = _build_nc()
    return _nc_cache["nc"]


def _host_prep_weights(integral_coeffs):
    """coeffs [G,G,C] -> wl [128, NGROUPS + NGROUPS*32] fp16 (lam | wblk).

    Per group g, tile r: wblk[32r + 8q + i, 32g + 8q + k] holds, for cell
    ct = 16g + 4r + q, either V_ct[i,k] (EM groups) or S_ct[i,k]
    (AD groups).  lam[32c + 8q + k, g] holds the matching psum-partition
    weight for mm2: lam_ct[k] with r=(c-sigma)%4 for EM groups; unused
    (1.0) for AD groups.
    """
    ii, jj = np.triu_indices(P)
    wq = integral_coeffs.reshape(CELLS, len(ii)).astype(np.float64)
    S = np.zeros((CELLS, P, P), np.float64)
    np.add.at(S, (slice(None), ii, jj), 0.5 * wq)
    np.add.at(S, (slice(None), jj, ii), 0.5 * wq)
    lam, V = np.linalg.eigh(S)

    S_p = np.zeros((CELLS_PAD, P, P))
    S_p[:CELLS] = S
    lam_p = np.zeros((CELLS_PAD, P))
    lam_p[:CELLS] = lam
    V_p = np.zeros((CELLS_PAD, P, P))
    V_p[:CELLS] = V

    wblk = np.zeros((PARTS, NGROUPS * 32), np.float32)
    lamt = np.zeros((PARTS, NGROUPS), np.float32)
    for g in range(NGROUPS):
        eigen = _group_form(g) == "E"
        M = V_p if eigen else S_p
        for r in range(4):
            for q in range(4):
                ct = 16 * g + 4 * r + q
                wblk[32 * r + 8 * q : 32 * r + 8 * q + 8,
                     32 * g + 8 * q : 32 * g + 8 * q + 8] = M[ct]
        if eigen:
            for c in range(4):
                r = (c - _sigma(g)) % 4
                for q in range(4):
                    ct = 16 * g + 4 * r + q
                    lamt[32 * c + 8 * q : 32 * c + 8 * q + 8, g] = lam_p[ct]
        else:
            lamt[:, g] = 1.0
    wl = np.concatenate([lamt, wblk], axis=1).astype(np.float16)
    return np.ascontiguousarray(wl)


def _host_prep_param(param_tensor):
    """param [B,G,G,P] f32 -> list of per-core [128, NGROUPS*NB] fp16 arrays."""
    flat = param_tensor.reshape(B, CELLS * P)
    out = []
    for c in range(NCORES):
        shard = flat[c * NB : (c + 1) * NB]
        pad = np.zeros((NB, CELLS_PAD * P), np.float32)
        pad[:, : CELLS * P] = shard
        # (b, g, p) -> (p, g, b)
        pt = (
            pad.reshape(NB, NGROUPS, PARTS)
            .transpose(2, 1, 0)
            .reshape(PARTS, NGROUPS * NB)
            .astype(np.float16)
        )
        out.append(np.ascontiguousarray(pt))
    return out


def _run(param_tensor, integral_coeffs, trace=False, **run_kwargs):
    from concourse.bass_utils import run_bass_kernel_spmd

    nc = _get_nc()
    wl = _host_prep_weights(np.asarray(integral_coeffs, np.float32))
    pts = _host_prep_param(np.asarray(param_tensor, np.float32))
    in_maps = [{"pt": pts[c], "wl": wl} for c in range(NCORES)]
    res = run_bass_kernel_spmd(
        nc, in_maps, core_ids=list(range(NCORES)), trace=trace, **run_kwargs
    )
    out = np.concatenate(
        [
            (
                res.results[c]["out"].sum(axis=0)
                + res.results[c]["zacd"].astype(np.float32).sum(axis=0)
                + res.results[c]["zacg"].astype(np.float32).sum(axis=0)
            ).reshape(NB)
            for c in range(NCORES)
        ]
    ).astype(np.float32)
    return out, res


def kernel(param_tensor, integral_coeffs):
    out, _ = _run(param_tensor, integral_coeffs)
    return out
